# revision 1
# baseline (speedup 1.0000x reference)
"""Trainium2 Bass kernel for an edge-weighted two-layer sparse MLP (QBAF).

Math (identical to the gather/segment_sum reference):
    out = sigmoid(x @ W1 + b1) @ W2 + b2
where W1 [2048, 1024] / W2 [1024, 8] are densified on host from the
sparse edge lists (scatter-add of per-edge weights; duplicate edges
accumulate, exactly like segment_sum).

Sharding: data-parallel over the batch dim — 8 cores x 512 rows each.
Weights/biases are replicated (W1 is 4 MB in bf16).

On-device layout: everything transposed so the contraction dim sits on
the SBUF partition axis:
    hT = sigmoid(sum_k W1_k^T-slices @ xT_k + b1)   [1024, 512] tiles
    outT = sum_m W2_m^T @ hT_m + b2                 [8, 512]
Layer 1 runs in bf16 (inputs) with fp32 PSUM accumulation; layer 2 in
float32r. Loop order is k-outer / m-inner so all 8 PSUM banks
accumulate while the DMAs stream in, keeping the PE continuously busy
from the first tile. xT and W1 are fused row-block-wise into one DRAM
tensor so each k-step is a single large DMA (per-DMA issue on the sync
sequencer is ~0.5-0.8 us and would otherwise pace the whole kernel).
"""

import sys

import numpy as np

if "/opt/trn_rl_repo" not in sys.path:
    sys.path.insert(0, "/opt/trn_rl_repo")

B = 4096
F = 2048
N1 = 1024
NT = 8
NCORES = 8
BSH = B // NCORES  # 512 batch rows per core
P = 128
K1 = F // P  # 16 contraction tiles, layer 1
M1 = N1 // P  # 8 neuron tiles
K2 = N1 // P  # 8 contraction tiles, layer 2
LXW = BSH + N1  # fused row width: [xT | W1]

# Layer-1 matmul input dtype: bf16 halves DMA traffic and runs the PE at
# 1 cycle/row. Flip to False for float32r layer-1 inputs.
L1_BF16 = True

_CACHE = {}


def _build(l1_bf16=L1_BF16):
    """Trace the Bass/Tile program. Returns the Bass object (uncompiled --
    run_bass_kernel_spmd / bass2jax handles BIR lowering + neuronxcc)."""
    import concourse.bass as bass
    import concourse.mybir as mybir
    import concourse.tile as tile

    dt = mybir.dt
    l1_dt = dt.bfloat16 if l1_bf16 else dt.float32r

    nc = bass.Bass()
    lx = nc.declare_dram_parameter("lx", [F, LXW], l1_dt, isOutput=False)
    w2p = nc.declare_dram_parameter("w2p", [P, K2 * NT], dt.float32r, isOutput=False)
    cn = nc.declare_dram_parameter("cn", [P, M1 + 1], dt.float32, isOutput=False)
    outT = nc.declare_dram_parameter("outT", [NT, BSH], dt.float32, isOutput=True)

    with tile.TileContext(nc) as tc:
        with (
            tc.tile_pool(name="consts", bufs=1) as consts,
            tc.tile_pool(name="lxp", bufs=K1) as lxp,
            tc.tile_pool(name="hp", bufs=M1) as hp,
            tc.tile_pool(name="outp", bufs=1) as outp,
            tc.tile_pool(name="ps", bufs=8, space="PSUM") as ps,
        ):
            # First fused input tiles go out before anything else so the PE
            # can start; descriptor writing is ~1us per DMA per sequencer, so
            # alternate between the two HWDGE rings (SP and ACT) to halve the
            # serial issue time. The tiny const loads ride behind them.
            lxts = []
            for k in range(K1):
                t = lxp.tile([P, LXW], l1_dt, tag="lx", name=f"lx{k}")
                eng = nc.sync if k % 2 == 0 else nc.scalar
                eng.dma_start(out=t[:], in_=lx[k * P : (k + 1) * P, :])
                lxts.append(t)
                if k == 1:
                    w2s = consts.tile([P, K2 * NT], dt.float32r, tag="w2", name="w2s")
                    nc.scalar.dma_start(out=w2s[:], in_=w2p[:])
                    cns = consts.tile([P, M1 + 1], dt.float32, tag="cn", name="cns")
                    nc.scalar.dma_start(out=cns[:], in_=cn[:])
                    # ACT pre-observes the cns DMA semaphore here (off the
                    # critical path); hw allows only ONE wait per ACT
                    # instruction, and the first sigmoid already needs the
                    # PE wait.
                    scr = consts.tile([P, 1], dt.float32, tag="scr", name="scr")
                    nc.scalar.activation(
                        scr[:], cns[:, 0:1], mybir.ActivationFunctionType.Copy
                    )

            # Layer 1: 8 PSUM accumulation groups (one per neuron tile),
            # k-outer so group m only waits on fused tiles k<=current.
            accs = [
                ps.tile([P, BSH], dt.float32, tag="acc", name=f"acc{m}")
                for m in range(M1)
            ]

            # HAM warm-up: the PE clock is gated to 1.2 GHz until ~3.4us of
            # sustained activity. The PE is otherwise idle while the first
            # lx tiles stream in (~10us), so burn that window on dummy
            # matmuls over a memset scratch tile. They write acc bank 0 as
            # self-contained start/stop groups; the real k=0 matmul below
            # resets the bank (start=True), so results are never observed.
            wsc = consts.tile([P, BSH], l1_dt, tag="wsc", name="wsc")
            nc.gpsimd.memset(wsc[:], 0.0)
            for i in range(10):
                nc.tensor.matmul(
                    accs[0][:], wsc[:, 0:P], wsc[:], start=True, stop=True
                )

            for k in range(K1):
                for m in range(M1):
                    nc.tensor.matmul(
                        accs[m][:],
                        lxts[k][:, BSH + m * P : BSH + (m + 1) * P],
                        lxts[k][:, 0:BSH],
                        start=(k == 0),
                        stop=(k == K1 - 1),
                    )

            # sigmoid(acc + b1) -> hT tiles, written as float32r so the
            # layer-2 matmul can consume them in full-rate fp32 mode.
            hts = []
            for m in range(M1):
                ht = hp.tile([P, BSH], dt.float32r, tag="h", name=f"h{m}")
                nc.scalar.activation(
                    ht[:],
                    accs[m][:],
                    mybir.ActivationFunctionType.Sigmoid,
                    bias=cns[:, m : m + 1],
                    scale=1.0,
                )
                hts.append(ht)

            # Layer 2: one [8, 512] accumulation group. 9th 'acc' tile in an
            # 8-buf pool -> reuses the bank freed by the first sigmoid.
            acc2 = ps.tile([P, BSH], dt.float32, tag="acc", name="acc2")
            for m in range(M1):
                nc.tensor.matmul(
                    acc2[:NT, :],
                    w2s[:, m * NT : (m + 1) * NT],
                    hts[m][:],
                    start=(m == 0),
                    stop=(m == M1 - 1),
                )

            outs = outp.tile([NT, BSH], dt.float32, tag="out", name="outs")
            nc.scalar.activation(
                outs[:],
                acc2[:NT, :],
                mybir.ActivationFunctionType.Identity,
                bias=cns[0:NT, M1 : M1 + 1],
                scale=1.0,
            )
            # SWDGE (gpsimd) queue: unused so far, so this carries only the
            # ACT data-dep wait -- hw allows a single wait per instruction,
            # and a sync-queue DMA would also need its lane-reuse wait.
            nc.gpsimd.dma_start(out=outT[:], in_=outs[:])

    return nc


def _strip_start_barrier(nc):
    """Drop the start-of-kernel all-engine drain + EVSEM barrier that Tile
    emits in the 'main' block (~1.5-2us). All Tile semaphores start at 0
    (and this kernel's tail clears them again), and every cross-engine
    dependency inside the kernel is already semaphore-guarded, so engines
    may enter the kernel body unsynchronized."""
    for fn in nc.m.functions:
        for bb in fn.blocks:
            if bb.name == "main":
                bb.instructions = [
                    i
                    for i in bb.instructions
                    if type(i).__name__ not in ("InstDrain", "InstEventSemaphore")
                ]


def _legalize_single_wait(nc):
    """This neuronxcc build allows at most ONE sync wait per instruction
    (setupSyncWait: 'Too many sync wait commands'). Tile emits multi-wait
    instructions (notably the kernel-tail Drain, which waits on every
    engine + DMA lane). Split the extras onto same-engine no-ops placed
    immediately before the instruction."""
    import bass_rust

    for fn in nc.m.functions:
        for bb in fn.blocks:
            out, changed = [], False
            for ins in bb.instructions:
                si = ins.sync_info
                waits = list(si.on_wait) if si is not None else []
                if len(waits) > 1:
                    for i, w in enumerate(waits[:-1]):
                        out.append(
                            bass_rust.InstNoOp(
                                name=f"{ins.name}-sw{i}",
                                engine=ins.engine,
                                ins=[],
                                outs=[],
                                sync_info=bass_rust.SyncInfo(
                                    on_wait=[w], on_update=[]
                                ),
                            )
                        )
                    ins.sync_info = bass_rust.SyncInfo(
                        on_wait=[waits[-1]], on_update=list(si.on_update)
                    )
                    changed = True
                out.append(ins)
            if changed:
                bb.instructions = out


def _densify(w, rows_in, cols_out, n_in, n_out):
    dense = np.zeros((n_in, n_out), np.float32)
    np.add.at(dense, (np.asarray(rows_in), np.asarray(cols_out)), np.asarray(w))
    return dense


def _prep_inputs(x, w1, b1, w2, b2, conn1_out, conn1_in, conn2_out, conn2_in, l1_bf16):
    import ml_dtypes

    ldt = ml_dtypes.bfloat16 if l1_bf16 else np.float32
    x = np.asarray(x, dtype=np.float32)
    W1 = _densify(w1, conn1_in, conn1_out, F, N1).astype(ldt)
    W2 = _densify(w2, conn2_in, conn2_out, N1, NT)
    # w2 packed k-major: w2p[p, k*NT + t] = W2[k*P + p, t]
    w2p = np.ascontiguousarray(
        W2.reshape(K2, P, NT).transpose(1, 0, 2).reshape(P, K2 * NT)
    )
    # consts: cols 0..M1-1 = b1 tiles, col M1 = b2 (on partitions 0..NT-1)
    cn = np.zeros((P, M1 + 1), np.float32)
    cn[:, :M1] = np.asarray(b1, np.float32).reshape(M1, P).T
    cn[:NT, M1] = np.asarray(b2, np.float32)
    xl = x.astype(ldt)
    in_maps = []
    for c in range(NCORES):
        lx = np.empty((F, LXW), ldt)
        lx[:, :BSH] = xl[c * BSH : (c + 1) * BSH, :].T
        lx[:, BSH:] = W1
        in_maps.append({"lx": lx, "w2p": w2p, "cn": cn})
    return in_maps


def _run(inputs, l1_bf16=L1_BF16, trace=False, **run_kwargs):
    """Build (cached), run on the 8 NeuronCores, gather. Returns
    (out [4096, 8] float32, BassKernelResults)."""
    from concourse.bass_utils import run_bass_kernel_spmd

    key = ("nc", l1_bf16)
    if key not in _CACHE:
        nc = _build(l1_bf16)
        # HW-only passes: CoreSim can't schedule post-hoc IR edits, but
        # the split waits are semantically identical for the compiler.
        _strip_start_barrier(nc)
        _legalize_single_wait(nc)
        _CACHE[key] = nc
    nc = _CACHE[key]

    in_maps = _prep_inputs(**inputs, l1_bf16=l1_bf16)
    res = run_bass_kernel_spmd(
        nc, in_maps, list(range(NCORES)), trace=trace, **run_kwargs
    )
    out = np.empty((B, NT), np.float32)
    for c in range(NCORES):
        out[c * BSH : (c + 1) * BSH, :] = res.results[c]["outT"].T
    return out, res


def kernel(**inputs):
    out, _ = _run(inputs)
    return out



# revision 4
# speedup vs baseline: 1.2571x; 1.2571x over previous
"""Trainium2 Bass kernel for an edge-weighted two-layer sparse MLP (QBAF).

Math (identical to the gather/segment_sum reference):
    out = sigmoid(x @ W1 + b1) @ W2 + b2
with W1 [2048, 1024] / W2 [1024, 8] densified on host from the sparse
edge lists (scatter-add, duplicates accumulate like segment_sum).

Sharding: data-parallel over batch — 8 cores x 512 rows each; weights
replicated.

Per-core plan (PE-roofline oriented):
  - Layer 1 mixed precision: first N8=12 contraction k-tiles run as 6
    fp8(e4m3) DoubleRow pair-tiles (2 k-tiles per matmul at the 2x fp8
    rate), the remaining 4 k-tiles in fp16. Host pre-scales x by 16 and
    W1 by 64 (keeps e4m3 out of subnormals; exact powers of two) so all
    contributions accumulate at one PSUM scale; the sigmoid activation
    applies scale=1/1024 plus the b1 bias.
  - Batch is split into two 256-col halves A/B that share the 8 PSUM
    banks (A in cols 0:256, B in 256:512). A's k0 matmul uses
    start=True, which marks the whole 2KB bank pending-zero; B's k0
    matmul then uses start=False and zero-fills its half. This lets
    half A's sigmoids/L2 overlap half B's layer-1 matmuls, halving the
    end-of-kernel Scalar (ACT) sigmoid chain.
  - Layer 2 in fp16 at full PE rate; its [8, 256] accumulators reuse
    dead half-bank space (freed once the sigmoid consumed it) via a
    gpsimd memzero + start=False accumulation.
  - All large DMAs are issued sequentially on the sync HWDGE ring so
    tiles arrive in exactly consumption order; consts ride the scalar
    ring; outputs return on sync.
"""

import sys

import numpy as np

if "/opt/trn_rl_repo" not in sys.path:
    sys.path.insert(0, "/opt/trn_rl_repo")

B = 4096
F = 2048
N1 = 1024
NT = 8
NCORES = 8
BSH = B // NCORES  # 512 batch rows per core
HB = BSH // 2  # 256-col batch half
P = 128
K1 = F // P  # 16 contraction k-tiles
M1 = N1 // P  # 8 neuron tiles
K2 = N1 // P  # 8 contraction tiles, layer 2
LXW = BSH + N1  # fused row width: [xT | W1] = 1536

N8 = 12  # k-tiles in fp8 (even; rest fp16). Host-sim rel err 0.0160.
NPAIR = N8 // 2
N16 = K1 - N8
SX = 16.0  # x pre-scale
SW = 64.0  # W1 pre-scale
SINV = 1.0 / (SX * SW)
NWARM = 8  # PE clock-ramp warmup matmuls

_CACHE = {}


def _build(n8=N8, nwarm=NWARM):
    import concourse.bass as bass
    import concourse.mybir as mybir
    import concourse.tile as tile

    dt = mybir.dt
    DR = mybir.MatmulPerfMode.DoubleRow
    npair = n8 // 2
    n16 = K1 - n8

    nc = bass.Bass()
    lx8 = nc.declare_dram_parameter("lx8", [npair * P, 2 * LXW], dt.float8e4, isOutput=False)
    lxh = nc.declare_dram_parameter("lxh", [n16 * P, LXW], dt.float16, isOutput=False)
    w2p = nc.declare_dram_parameter("w2p", [P, K2 * NT], dt.float16, isOutput=False)
    cn = nc.declare_dram_parameter("cn", [P, M1 + 1], dt.float32, isOutput=False)
    outT = nc.declare_dram_parameter("outT", [NT, BSH], dt.float32, isOutput=True)

    with tile.TileContext(nc) as tc:
        with (
            tc.tile_pool(name="consts", bufs=1) as consts,
            tc.tile_pool(name="lx8p", bufs=max(npair, 1)) as lx8p,
            tc.tile_pool(name="lxhp", bufs=max(n16, 1)) as lxhp,
            tc.tile_pool(name="hp", bufs=2 * M1) as hp,
            tc.tile_pool(name="outp", bufs=2) as outp,
            tc.tile_pool(name="ps", bufs=8, space="PSUM") as ps,
        ):
            # --- DMAs: strictly sequential big tiles on the sync ring so
            # arrival order == PE consumption order; consts on scalar.
            t8s = []
            for j in range(npair):
                t = lx8p.tile([P, 2, LXW], dt.float8e4, tag="lx8", name=f"lx8_{j}")
                nc.sync.dma_start(out=t[:], in_=lx8[j * P : (j + 1) * P, :])
                t8s.append(t)
                if j == 0:
                    w2s = consts.tile([P, K2 * NT], dt.float16, tag="w2", name="w2s")
                    nc.scalar.dma_start(out=w2s[:], in_=w2p[:])
                    cns = consts.tile([P, M1 + 1], dt.float32, tag="cn", name="cns")
                    nc.scalar.dma_start(out=cns[:], in_=cn[:])
                    # ACT pre-observes the cns DMA semaphore off the critical
                    # path (hw allows one wait per ACT instruction, and the
                    # first sigmoid already needs the PE wait).
                    scr = consts.tile([P, 1], dt.float32, tag="scr", name="scr")
                    nc.scalar.activation(
                        scr[:], cns[:, 0:1], mybir.ActivationFunctionType.Copy
                    )
            ths = []
            for k in range(n16):
                t = lxhp.tile([P, LXW], dt.float16, tag="lxh", name=f"lxh_{k}")
                nc.sync.dma_start(out=t[:], in_=lxh[k * P : (k + 1) * P, :])
                ths.append(t)

            # --- 8 shared PSUM banks: half A in cols 0:HB, half B in HB:BSH.
            accs = [
                ps.tile([P, BSH], dt.float32, tag="acc", name=f"acc{m}")
                for m in range(M1)
            ]

            # --- HAM warm-up: PE clock is gated ~1.2 GHz until a few us of
            # sustained activity. Scratch tile memset rides the otherwise
            # idle Vector engine so warmups start right after the preamble.
            wsc = consts.tile([P, HB], dt.float16, tag="wsc", name="wsc")
            nc.vector.memset(wsc[:], 0.0)
            for _ in range(nwarm):
                nc.tensor.matmul(
                    accs[0][:, 0:HB], wsc[:, 0:P], wsc[:], start=True, stop=True,
                    skip_group_check=True,
                )

            # --- Layer 1, fp8 phase: halves interleaved per pair-tile.
            # A j0 start=True marks the whole bank pending-zero; B j0
            # start=False zero-fills its half (hardware PSUM zero-region
            # semantics, ZERO_REGION = full 2KB bank).
            for j in range(npair):
                for hoff in (0, HB):
                    for m in range(M1):
                        nc.tensor.matmul(
                            accs[m][:, hoff : hoff + HB],
                            t8s[j][:, :, BSH + m * P : BSH + (m + 1) * P],
                            t8s[j][:, :, hoff : hoff + HB],
                            start=(j == 0 and hoff == 0),
                            stop=False,
                            perf_mode=DR,
                            skip_group_check=True,
                        )

            # --- fp16 phase, half A completes first so its sigmoids overlap
            # half B's remaining matmuls.
            for k in range(n16):
                for m in range(M1):
                    nc.tensor.matmul(
                        accs[m][:, 0:HB],
                        ths[k][:, BSH + m * P : BSH + (m + 1) * P],
                        ths[k][:, 0:HB],
                        start=False,
                        stop=(k == n16 - 1),
                        skip_group_check=True,
                    )

            hAs = []
            for m in range(M1):
                ht = hp.tile([P, HB], dt.float16, tag="h", name=f"hA{m}")
                nc.scalar.activation(
                    ht[:],
                    accs[m][:, 0:HB],
                    mybir.ActivationFunctionType.Sigmoid,
                    bias=cns[:, m : m + 1],
                    scale=SINV,
                )
                hAs.append(ht)

            for k in range(n16):
                for m in range(M1):
                    nc.tensor.matmul(
                        accs[m][:, HB:BSH],
                        ths[k][:, BSH + m * P : BSH + (m + 1) * P],
                        ths[k][:, HB:BSH],
                        start=False,
                        stop=(k == n16 - 1),
                        skip_group_check=True,
                    )

            # --- Layer 2 half A into dead half-bank space (sigmoid A0
            # already consumed acc[0][:, 0:HB]); gpsimd zeroes the [8, HB]
            # region, matmuls accumulate with start=False on top.
            nc.vector.memset(accs[0][0:NT, 0:HB], 0.0)
            for m in range(M1):
                nc.tensor.matmul(
                    accs[0][0:NT, 0:HB],
                    w2s[:, m * NT : (m + 1) * NT],
                    hAs[m][:],
                    start=False,
                    stop=(m == M1 - 1),
                    skip_group_check=True,
                )
            outsA = outp.tile([NT, HB], dt.float32, tag="out", name="outsA")
            nc.scalar.activation(
                outsA[:],
                accs[0][0:NT, 0:HB],
                mybir.ActivationFunctionType.Identity,
                bias=cns[0:NT, M1 : M1 + 1],
                scale=1.0,
            )
            nc.sync.dma_start(out=outT[:, 0:HB], in_=outsA[:])

            # --- Half B sigmoids + L2 + out.
            hBs = []
            for m in range(M1):
                ht = hp.tile([P, HB], dt.float16, tag="h", name=f"hB{m}")
                nc.scalar.activation(
                    ht[:],
                    accs[m][:, HB:BSH],
                    mybir.ActivationFunctionType.Sigmoid,
                    bias=cns[:, m : m + 1],
                    scale=SINV,
                )
                hBs.append(ht)
            nc.vector.memset(accs[0][0:NT, HB:BSH], 0.0)
            for m in range(M1):
                nc.tensor.matmul(
                    accs[0][0:NT, HB:BSH],
                    w2s[:, m * NT : (m + 1) * NT],
                    hBs[m][:],
                    start=False,
                    stop=(m == M1 - 1),
                    skip_group_check=True,
                )
            outsB = outp.tile([NT, HB], dt.float32, tag="out", name="outsB")
            nc.scalar.activation(
                outsB[:],
                accs[0][0:NT, HB:BSH],
                mybir.ActivationFunctionType.Identity,
                bias=cns[0:NT, M1 : M1 + 1],
                scale=1.0,
            )
            nc.sync.dma_start(out=outT[:, HB:BSH], in_=outsB[:])

    return nc


def _strip_start_barrier(nc):
    """Drop the start-of-kernel all-engine drain + EVSEM barrier Tile emits
    in the 'main' block (~1.5-2us). All Tile semaphores start at 0 and every
    cross-engine dependency is already semaphore-guarded."""
    for fn in nc.m.functions:
        for bb in fn.blocks:
            if bb.name == "main":
                bb.instructions = [
                    i
                    for i in bb.instructions
                    if type(i).__name__ not in ("InstDrain", "InstEventSemaphore")
                ]


def _legalize_single_wait(nc):
    """This neuronxcc build allows at most ONE sync wait per instruction.
    Split extras onto same-engine no-ops placed immediately before."""
    import bass_rust

    for fn in nc.m.functions:
        for bb in fn.blocks:
            out, changed = [], False
            for ins in bb.instructions:
                si = ins.sync_info
                waits = list(si.on_wait) if si is not None else []
                if len(waits) > 1:
                    for i, w in enumerate(waits[:-1]):
                        out.append(
                            bass_rust.InstNoOp(
                                name=f"{ins.name}-sw{i}",
                                engine=ins.engine,
                                ins=[],
                                outs=[],
                                sync_info=bass_rust.SyncInfo(
                                    on_wait=[w], on_update=[]
                                ),
                            )
                        )
                    ins.sync_info = bass_rust.SyncInfo(
                        on_wait=[waits[-1]], on_update=list(si.on_update)
                    )
                    changed = True
                out.append(ins)
            if changed:
                bb.instructions = out


def _densify(w, rows_in, cols_out, n_in, n_out):
    dense = np.zeros((n_in, n_out), np.float32)
    np.add.at(dense, (np.asarray(rows_in), np.asarray(cols_out)), np.asarray(w))
    return dense


def _prep_inputs(x, w1, b1, w2, b2, conn1_out, conn1_in, conn2_out, conn2_in, n8=N8):
    import ml_dtypes

    f8 = ml_dtypes.float8_e4m3fn
    npair = n8 // 2
    x = np.asarray(x, dtype=np.float32)
    W1 = _densify(w1, conn1_in, conn1_out, F, N1)
    W2 = _densify(w2, conn2_in, conn2_out, N1, NT).astype(np.float16)
    # w2 packed k-major: w2p[p, m*NT + t] = W2[m*P + p, t]
    w2p = np.ascontiguousarray(
        W2.reshape(K2, P, NT).transpose(1, 0, 2).reshape(P, K2 * NT)
    )
    cn = np.zeros((P, M1 + 1), np.float32)
    cn[:, :M1] = np.asarray(b1, np.float32).reshape(M1, P).T
    cn[:NT, M1] = np.asarray(b2, np.float32)

    W1s = SW * W1  # [2048, 1024], scaled
    xs = SX * x  # [4096, 2048], scaled
    in_maps = []
    for c in range(NCORES):
        xT = np.ascontiguousarray(xs[c * BSH : (c + 1) * BSH, :].T)  # [F, BSH]
        V = np.concatenate([xT, W1s], axis=1)  # [F, LXW] fp32, scaled
        # fp8 pair-tiles: row (j*128+p) = [fused(256j+p) | fused(256j+128+p)]
        v8 = V[: n8 * P].astype(f8)
        lx8 = np.ascontiguousarray(
            v8.reshape(npair, 2, P, LXW).transpose(0, 2, 1, 3).reshape(npair * P, 2 * LXW)
        )
        lxh = np.ascontiguousarray(V[n8 * P :].astype(np.float16))
        in_maps.append({"lx8": lx8, "lxh": lxh, "w2p": w2p, "cn": cn})
    return in_maps


def _run(inputs, l1_bf16=True, trace=False, n8=N8, nwarm=NWARM, **run_kwargs):
    """Build (cached), run on the 8 NeuronCores, gather. Returns
    (out [4096, 8] float32, BassKernelResults). l1_bf16 is accepted for
    test-harness compat and ignored (layer 1 is mixed fp8/fp16)."""
    from concourse.bass_utils import run_bass_kernel_spmd

    key = ("nc", n8, nwarm)
    if key not in _CACHE:
        nc = _build(n8, nwarm)
        _strip_start_barrier(nc)
        _legalize_single_wait(nc)
        _CACHE[key] = nc
    nc = _CACHE[key]

    in_maps = _prep_inputs(**inputs, n8=n8)
    res = run_bass_kernel_spmd(
        nc, in_maps, list(range(NCORES)), trace=trace, **run_kwargs
    )
    out = np.empty((B, NT), np.float32)
    for c in range(NCORES):
        out[c * BSH : (c + 1) * BSH, :] = res.results[c]["outT"].T
    return out, res


def kernel(**inputs):
    out, _ = _run(inputs)
    return out


# revision 5
# speedup vs baseline: 1.4283x; 1.1362x over previous
"""Trainium2 Bass kernel for an edge-weighted two-layer sparse MLP (QBAF).

Math (identical to the gather/segment_sum reference):
    out = sigmoid(x @ W1 + b1) @ W2 + b2
with W1 [2048, 1024] / W2 [1024, 8] densified on host from the sparse
edge lists (scatter-add, duplicates accumulate like segment_sum).

Sharding: data-parallel over batch — 8 cores x 512 rows each; weights
replicated.

Per-core plan (PE-roofline oriented):
  - Layer 1 mixed precision: the first N8=12 contraction k-tiles run as
    6 fp8(e4m3) DoubleRow pair-tiles (2 k-tiles per matmul at the 2x
    fp8 rate), the remaining 4 k-tiles in fp16 at full rate. Host
    pre-scales x by 16 and W1 by 64 (keeps e4m3 away from subnormals;
    exact powers of two) so both phases accumulate at one PSUM scale;
    the sigmoid applies scale=1/1024 plus the b1 bias. Host-simulated
    rel err: 0.0160 (threshold 2e-2).
  - fp8 phase is k-outer (tiles consumed in DMA arrival order); the
    fp16 phase is m-outer so acc[m] banks complete staggered and the
    full-width sigmoid chain on ACT overlaps the tail of layer 1.
  - Layer 2 in fp16 at full rate chases the sigmoids; one bias-add and
    one [8, 512] out DMA on the sync ring finish the kernel.
  - Large DMAs are issued sequentially on the sync HWDGE ring so tiles
    arrive in exactly consumption order; consts ride the scalar ring.
  - Post-build IR passes strip Tile's start barrier, register-init
    moves, dead const memsets, and all end-of-kernel drains except the
    sync drain that guards out-DMA completion.
"""

import sys

import numpy as np

if "/opt/trn_rl_repo" not in sys.path:
    sys.path.insert(0, "/opt/trn_rl_repo")

B = 4096
F = 2048
N1 = 1024
NT = 8
NCORES = 8
BSH = B // NCORES  # 512 batch rows per core
P = 128
K1 = F // P  # 16 contraction k-tiles
M1 = N1 // P  # 8 neuron tiles
K2 = N1 // P  # 8 contraction tiles, layer 2
LXW = BSH + N1  # fused row width: [xT | W1] = 1536

N8 = 12  # k-tiles in fp8 (even; rest fp16)
SX = 16.0  # x pre-scale
SW = 64.0  # W1 pre-scale
SINV = 1.0 / (SX * SW)
NWARM = 14  # PE clock-ramp warmup matmuls (256-col, ~214ns each gated)

_CACHE = {}


def _build(n8=N8, nwarm=NWARM):
    import concourse.bass as bass
    import concourse.mybir as mybir
    import concourse.tile as tile

    dt = mybir.dt
    DR = mybir.MatmulPerfMode.DoubleRow
    npair = n8 // 2
    n16 = K1 - n8

    nc = bass.Bass()
    lx8 = nc.declare_dram_parameter("lx8", [npair * P, 2 * LXW], dt.float8e4, isOutput=False)
    lxh = nc.declare_dram_parameter("lxh", [n16 * P, LXW], dt.float16, isOutput=False)
    w2p = nc.declare_dram_parameter("w2p", [P, K2 * NT], dt.float16, isOutput=False)
    cn = nc.declare_dram_parameter("cn", [P, M1 + 1], dt.float32, isOutput=False)
    outT = nc.declare_dram_parameter("outT", [NT, BSH], dt.float32, isOutput=True)

    with tile.TileContext(nc) as tc:
        with (
            tc.tile_pool(name="consts", bufs=1) as consts,
            tc.tile_pool(name="lx8p", bufs=max(npair, 1)) as lx8p,
            tc.tile_pool(name="lxhp", bufs=max(n16, 1)) as lxhp,
            tc.tile_pool(name="hp", bufs=M1) as hp,
            tc.tile_pool(name="outp", bufs=1) as outp,
            tc.tile_pool(name="ps", bufs=8, space="PSUM") as ps,
        ):
            # --- DMAs: big tiles strictly sequential on the sync ring so
            # arrival order == PE consumption order; consts on scalar.
            t8s = []
            for j in range(npair):
                t = lx8p.tile([P, 2, LXW], dt.float8e4, tag="lx8", name=f"lx8_{j}")
                nc.sync.dma_start(out=t[:], in_=lx8[j * P : (j + 1) * P, :])
                t8s.append(t)
                if j == 0:
                    w2s = consts.tile([P, K2 * NT], dt.float16, tag="w2", name="w2s")
                    nc.scalar.dma_start(out=w2s[:], in_=w2p[:])
                    cns = consts.tile([P, M1 + 1], dt.float32, tag="cn", name="cns")
                    nc.scalar.dma_start(out=cns[:], in_=cn[:])
                    # ACT pre-observes the cns DMA semaphore off the critical
                    # path (hw allows one wait per ACT instruction, and the
                    # first sigmoid already needs the PE wait).
                    scr = consts.tile([P, 1], dt.float32, tag="scr", name="scr")
                    nc.scalar.activation(
                        scr[:], cns[:, 0:1], mybir.ActivationFunctionType.Copy
                    )
            ths = []
            for k in range(n16):
                t = lxhp.tile([P, LXW], dt.float16, tag="lxh", name=f"lxh_{k}")
                nc.sync.dma_start(out=t[:], in_=lxh[k * P : (k + 1) * P, :])
                ths.append(t)

            accs = [
                ps.tile([P, BSH], dt.float32, tag="acc", name=f"acc{m}")
                for m in range(M1)
            ]

            # --- HAM warm-up: PE clock is gated ~1.2 GHz until a few us of
            # sustained activity; bridge the gap until the first tile lands.
            wsc = consts.tile([P, BSH // 2], dt.float16, tag="wsc", name="wsc")
            nc.vector.memset(wsc[:], 0.0)
            for _ in range(nwarm):
                nc.tensor.matmul(
                    accs[0][:, 0 : BSH // 2], wsc[:, 0:P], wsc[:],
                    start=True, stop=True, skip_group_check=True,
                )

            # --- Layer 1 fp8 phase, k-outer: 8 full-width DoubleRow matmuls
            # per pair-tile, one accumulation group per PSUM bank.
            for j in range(npair):
                for m in range(M1):
                    nc.tensor.matmul(
                        accs[m][:],
                        t8s[j][:, :, BSH + m * P : BSH + (m + 1) * P],
                        t8s[j][:, :, 0:BSH],
                        start=(j == 0),
                        stop=False,
                        perf_mode=DR,
                        skip_group_check=True,
                    )

            # --- fp16 phase, m-outer: acc[m] finishes after its 4 matmuls,
            # so sigmoids start ~6us before layer 1 ends and chase.
            hts = []
            for m in range(M1):
                for k in range(n16):
                    nc.tensor.matmul(
                        accs[m][:],
                        ths[k][:, BSH + m * P : BSH + (m + 1) * P],
                        ths[k][:, 0:BSH],
                        start=False,
                        stop=(k == n16 - 1),
                        skip_group_check=True,
                    )
                ht = hp.tile([P, BSH], dt.float16, tag="h", name=f"h{m}")
                nc.scalar.activation(
                    ht[:],
                    accs[m][:],
                    mybir.ActivationFunctionType.Sigmoid,
                    bias=cns[:, m : m + 1],
                    scale=SINV,
                )
                hts.append(ht)

            # --- Layer 2: full-width fp16, chases the sigmoid chain. acc2 is
            # the 9th psum tile -> reuses the bank freed by sigmoid 0.
            acc2 = ps.tile([P, BSH], dt.float32, tag="acc", name="acc2")
            for m in range(M1):
                nc.tensor.matmul(
                    acc2[:NT, :],
                    w2s[:, m * NT : (m + 1) * NT],
                    hts[m][:],
                    start=(m == 0),
                    stop=(m == M1 - 1),
                )
            outs = outp.tile([NT, BSH], dt.float32, tag="out", name="outs")
            nc.scalar.activation(
                outs[:],
                acc2[:NT, :],
                mybir.ActivationFunctionType.Identity,
                bias=cns[0:NT, M1 : M1 + 1],
                scale=1.0,
            )
            nc.sync.dma_start(out=outT[:], in_=outs[:])

    return nc


def _strip_start_barrier(nc):
    """Drop Tile's start-of-kernel all-engine drain + EVSEM barrier, the
    per-engine register-init moves (no hardware loops / predication in this
    kernel), and the never-read const-pool memsets from the 'main' block."""
    for fn in nc.m.functions:
        for bb in fn.blocks:
            if bb.name == "main":
                bb.instructions = [
                    i
                    for i in bb.instructions
                    if type(i).__name__
                    not in (
                        "InstDrain",
                        "InstEventSemaphore",
                        "InstRegisterMove",
                        "InstMemset",
                    )
                ]


def _slim_end_block(nc):
    """The Tile end block emits a Drain + barrier EventSemaphore pair per
    engine (paced by the slow gpsimd queue, ~2.4us of tail). Only the sync
    (SP) drain matters for correctness: it waits on the out-DMA completion
    semaphores so the NEFF cannot signal done with the transfer in flight.
    Each engine's own postamble runs in-order after its last real
    instruction, and every cross-engine semaphore a consumer waits on is
    produced earlier in its producer's queue, so the all-engine barrier is
    redundant."""
    from concourse import mybir

    for fn in nc.m.functions:
        for bb in fn.blocks:
            if bb.name.endswith("_end"):
                bb.instructions = [
                    i
                    for i in bb.instructions
                    if getattr(i, "engine", None) == mybir.EngineType.SP
                    and type(i).__name__ == "InstDrain"
                ]


def _legalize_single_wait(nc):
    """This neuronxcc build allows at most ONE sync wait per instruction.
    Split extras onto same-engine no-ops placed immediately before."""
    import bass_rust

    for fn in nc.m.functions:
        for bb in fn.blocks:
            out, changed = [], False
            for ins in bb.instructions:
                si = ins.sync_info
                waits = list(si.on_wait) if si is not None else []
                if len(waits) > 1:
                    for i, w in enumerate(waits[:-1]):
                        out.append(
                            bass_rust.InstNoOp(
                                name=f"{ins.name}-sw{i}",
                                engine=ins.engine,
                                ins=[],
                                outs=[],
                                sync_info=bass_rust.SyncInfo(
                                    on_wait=[w], on_update=[]
                                ),
                            )
                        )
                    ins.sync_info = bass_rust.SyncInfo(
                        on_wait=[waits[-1]], on_update=list(si.on_update)
                    )
                    changed = True
                out.append(ins)
            if changed:
                bb.instructions = out


def _densify(w, rows_in, cols_out, n_in, n_out):
    dense = np.zeros((n_in, n_out), np.float32)
    np.add.at(dense, (np.asarray(rows_in), np.asarray(cols_out)), np.asarray(w))
    return dense


def _prep_inputs(x, w1, b1, w2, b2, conn1_out, conn1_in, conn2_out, conn2_in, n8=N8):
    import ml_dtypes

    f8 = ml_dtypes.float8_e4m3fn
    npair = n8 // 2
    x = np.asarray(x, dtype=np.float32)
    W1 = _densify(w1, conn1_in, conn1_out, F, N1)
    W2 = _densify(w2, conn2_in, conn2_out, N1, NT).astype(np.float16)
    # w2 packed k-major: w2p[p, m*NT + t] = W2[m*P + p, t]
    w2p = np.ascontiguousarray(
        W2.reshape(K2, P, NT).transpose(1, 0, 2).reshape(P, K2 * NT)
    )
    cn = np.zeros((P, M1 + 1), np.float32)
    cn[:, :M1] = np.asarray(b1, np.float32).reshape(M1, P).T
    cn[:NT, M1] = np.asarray(b2, np.float32)

    W1s = SW * W1  # [2048, 1024], scaled
    xs = SX * x  # [4096, 2048], scaled
    in_maps = []
    for c in range(NCORES):
        xT = np.ascontiguousarray(xs[c * BSH : (c + 1) * BSH, :].T)  # [F, BSH]
        V = np.concatenate([xT, W1s], axis=1)  # [F, LXW] fp32, scaled
        # fp8 pair-tiles: row (j*128+p) = [fused(256j+p) | fused(256j+128+p)]
        v8 = V[: n8 * P].astype(f8)
        lx8 = np.ascontiguousarray(
            v8.reshape(npair, 2, P, LXW).transpose(0, 2, 1, 3).reshape(npair * P, 2 * LXW)
        )
        lxh = np.ascontiguousarray(V[n8 * P :].astype(np.float16))
        in_maps.append({"lx8": lx8, "lxh": lxh, "w2p": w2p, "cn": cn})
    return in_maps


def _run(inputs, l1_bf16=True, trace=False, n8=N8, nwarm=NWARM, **run_kwargs):
    """Build (cached), run on the 8 NeuronCores, gather. Returns
    (out [4096, 8] float32, BassKernelResults). l1_bf16 is accepted for
    test-harness compat and ignored (layer 1 is mixed fp8/fp16)."""
    from concourse.bass_utils import run_bass_kernel_spmd

    key = ("nc", n8, nwarm)
    if key not in _CACHE:
        nc = _build(n8, nwarm)
        _strip_start_barrier(nc)
        _slim_end_block(nc)
        _legalize_single_wait(nc)
        _CACHE[key] = nc
    nc = _CACHE[key]

    in_maps = _prep_inputs(**inputs, n8=n8)
    res = run_bass_kernel_spmd(
        nc, in_maps, list(range(NCORES)), trace=trace, **run_kwargs
    )
    out = np.empty((B, NT), np.float32)
    for c in range(NCORES):
        out[c * BSH : (c + 1) * BSH, :] = res.results[c]["outT"].T
    return out, res


def kernel(**inputs):
    out, _ = _run(inputs)
    return out
